# revision 16
# baseline (speedup 1.0000x reference)
"""Trainium2 Bass kernel for the confidence-based contrastive loss.

Distribution (8 NeuronCores, SPMD, no collectives):
  - The host owns the data-dependent sampling plan (exactly as the
    reference's host-side _plan does), gathers the sampled anchor and
    core-confidence pixels with one fancy-index, normalizes them, and
    reduces the tiny per-class core means + positive similarities.
  - The contrastive O(NA^2 * C) part runs on device: anchors are sharded
    8-ways over cores (512 anchors/class/core), the negative sets are
    replicated in fp8-e4m3 (x16 pre-scale keeps the unit vectors in the
    normal range; quantization error on the loss is ~1e-4).  Each core
    computes sim = anchors x negatives on PE (fp8 in, fp32 PSUM, 4-bank
    accumulation groups, stationary reused across the 8 moving chunks),
    exp(sim/tau') on ACT (2048-wide ops, single Exp table residency), and
    per-100-negative chunk sums on DVE as pairwise 100->50->25 bf16 folds
    (TensorTensor gets the 2x 16-bit mode; TensorReduce does not) plus a
    short segmented reduce.  The [128, 8*41] chunk-sum partials go back
    to the host, which applies exp(-pos/tau), log1p and the final mean.
"""

import sys

if "/opt/trn_rl_repo" not in sys.path:
    sys.path.insert(0, "/opt/trn_rl_repo")

import numpy as np
import ml_dtypes

import concourse.bass as bass
import concourse.tile as tile
from concourse import bacc, mybir
from concourse.bass_utils import run_bass_kernel_spmd

# ---- problem constants (must match reference.py) ----
TAU = 0.07
THRESHOLD = 0.8
SAMPLE_NUM = 4096
CHUNK = 100
_EPS_NORM = 1e-12

N_CORES = 8
H = W = 512
HW = H * W
C = 256
NA = SAMPLE_NUM          # anchors per class
ASL = NA // N_CORES      # 512 anchors per class per core
NIB = ASL // 128         # 4 anchor i-blocks of 128 per class per core
NBLK = 2 * NIB           # 8 blocks per core (cls-major)
NJS = NA // 512          # 8 moving 512-column negative chunks
NFULL = NA // CHUNK      # 40 full chunks
NCHUNK = NFULL + 1       # 41 (incl. 96-negative remainder chunk)

F32 = mybir.dt.float32
BF16 = mybir.dt.bfloat16
F8 = mybir.dt.float8e4
SCALE = 16.0
Alu = mybir.AluOpType
Act = mybir.ActivationFunctionType
Axis = mybir.AxisListType
BF16_NP = ml_dtypes.bfloat16
F8_NP = mybir.dt.np(mybir.dt.float8e4)


# ---------------------------------------------------------------------------
# host-side plan: verbatim replica of reference._plan (numpy, seed 0)
# ---------------------------------------------------------------------------
def _plan(input_logits, input_seg, seed=0):
    logits = np.asarray(input_logits)
    seg = np.asarray(input_seg)
    gm = seg == 1
    bm = seg == 0
    gc = logits[:, 1] * gm
    bc = logits[:, 0] * bm
    mgc = float(gc.sum() / (gm.sum() + 1e-8))
    mbc = float(bc.sum() / (bm.sum() + 1e-8))
    rng = np.random.default_rng(seed)

    def samp(mask, num):
        coords = np.argwhere(mask)
        if len(coords) > num:
            coords = coords[rng.permutation(len(coords))[:num]]
        return coords

    easy_g = max(1, int(SAMPLE_NUM * (1 - mgc))); hard_g = SAMPLE_NUM - easy_g
    easy_b = max(1, int(SAMPLE_NUM * (1 - mbc))); hard_b = SAMPLE_NUM - easy_b
    ge = samp((gc >= mgc) & gm, easy_g)
    gh = samp((gc < mgc) & gm, hard_g)
    be = samp((bc >= mbc) & bm, easy_b)
    bh = samp((bc < mbc) & bm, hard_b)
    return {
        "g_anchor": np.concatenate([ge, gh]),
        "b_anchor": np.concatenate([be, bh]),
        "g_core": np.argwhere((gc >= THRESHOLD) & gm),
        "b_core": np.argwhere((bc >= THRESHOLD) & bm),
        "n_bg": len(be) + len(bh),
    }


# ---------------------------------------------------------------------------
# device kernel: per core 1024 anchors x 2x4096 negs -> chunk-sum partials
# ---------------------------------------------------------------------------
def _build_kernel(nd=N_CORES):
    nc = bacc.Bacc("TRN2", target_bir_lowering=False, debug=False,
                   num_devices=nd)

    # amy: this core's anchors, channel-major halves; cols 0:512 g, 512:1024 b
    amy = nc.dram_tensor("amy", [2, 128, 2 * ASL], F8, kind="ExternalInput")
    # ball: all anchors (negative sets), index [h, cls]
    ball = nc.dram_tensor("ball", [2, 2, 128, NA], F8, kind="ExternalInput")
    # per-block per-chunk sums S = sum_j exp(sim_ij / tau)
    out = nc.dram_tensor("out", [128, NBLK * NCHUNK], BF16,
                         kind="ExternalOutput")

    with tile.TileContext(nc) as tc:
        with (
            tc.tile_pool(name="big", bufs=1) as big,
            tc.tile_pool(name="esb", bufs=2) as esbp,
            tc.tile_pool(name="small", bufs=2) as small,
            tc.tile_pool(name="acc", bufs=1) as accp,
            tc.tile_pool(name="pe", bufs=2, space="PSUM") as pe_pool,
        ):
            out_sb = accp.tile([128, NBLK * NCHUNK], BF16, tag="out_sb")

            # resident inputs; both channel-halves land with one DMA each
            # (in-AP "h p c -> p h c").  First-use negatives (cls=0 needs the
            # b-class set) stream in escalating chunks so PE starts early.
            amy_sb = big.tile([128, 2, 2 * ASL], F8, tag="amy")
            ball_sb = [big.tile([128, 2, NA], F8, tag=f"ball{negcls}",
                                name=f"ball_sb{negcls}")
                       for negcls in range(2)]
            amy_t = amy.ap().rearrange("a p c -> p a c")
            ball_t = [ball.ap()[:, negcls].rearrange("a p c -> p a c")
                      for negcls in range(2)]
            nc.sync.dma_start(amy_sb[:, :, 0:ASL], amy_t[:, :, 0:ASL])
            for lo, hi in ((0, 512), (512, 1024), (1024, 2048), (2048, 4096)):
                nc.sync.dma_start(ball_sb[1][:, :, lo:hi],
                                  ball_t[1][:, :, lo:hi])
            nc.sync.dma_start(amy_sb[:, :, ASL:2 * ASL],
                              amy_t[:, :, ASL:2 * ASL])
            for lo, hi in ((0, 2048), (2048, 4096)):
                nc.sync.dma_start(ball_sb[0][:, :, lo:hi],
                                  ball_t[0][:, :, lo:hi])

            for blk in range(NBLK):
                cls, ib = blk // NIB, blk % NIB
                negcls = 1 - cls
                icol = cls * ASL + ib * 128
                ocol = blk * NCHUNK
                esb = esbp.tile([128, NA], BF16, tag="esb")
                # block 0 uses escalating PSUM groups so the first exp fires
                # as soon as the first 512 negative columns have landed; the
                # last block splits its second half so the post-ACT DVE/DMA
                # tail is short
                if blk == 0:
                    groups = ((0, 1), (1, 2), (2, 4), (4, 8))
                elif blk == NBLK - 1:
                    groups = ((0, 4), (4, 6), (6, 8))
                else:
                    groups = ((0, 4), (4, 8))
                for glo, ghi in groups:
                    eps = pe_pool.tile([128, (ghi - glo) * 512], F32,
                                       tag="eps")
                    for h in range(2):  # h outer: stationary reuse across js
                        for js in range(glo, ghi):
                            nc.tensor.matmul(
                                eps[:, (js - glo) * 512:(js - glo + 1) * 512],
                                amy_sb[:, h, icol:icol + 128],
                                ball_sb[negcls][:, h, js * 512:
                                                (js + 1) * 512],
                                start=(h == 0), stop=(h == 1),
                            )
                    nc.scalar.activation(
                        esb[:, glo * 512:ghi * 512], eps[:],
                        Act.Exp, scale=1.0 / (SCALE * SCALE * TAU))
                # 100-col chunk sums.  DVE TensorReduce only runs at
                # 1 elem/cycle, but TensorTensor adds get the 2x bf16 mode —
                # fold 100->50->25 pairwise, then a short segmented reduce.
                # Each esb half (20 chunks) has its own chain so DVE overlaps
                # ACT within the block and the final-block tail stays short.
                ec = esb[:, 0:NFULL * CHUNK].rearrange(
                    "p (a b) -> p a b", b=CHUNK)
                e2 = small.tile([128, NFULL, 50], BF16, tag="e2")
                e3 = small.tile([128, NFULL, 25], BF16, tag="e3")
                r48 = small.tile([128, 48], BF16, tag="r48")
                r24 = small.tile([128, 24], BF16, tag="r24")
                chains = ((0, 20), (20, 30), (30, NFULL)) \
                    if blk == NBLK - 1 else ((0, 20), (20, NFULL))
                with nc.allow_low_precision(
                        reason="chunk sums of ~100 exp terms; bf16 "
                               "rounding is ~0.4% and averages out over "
                               "328k loss terms (tol 2e-2)"):
                    for chlo, chhi in chains:
                        nc.vector.tensor_tensor(
                            e2[:, chlo:chhi, :], ec[:, chlo:chhi, 0:50],
                            ec[:, chlo:chhi, 50:CHUNK], Alu.add)
                        nc.vector.tensor_tensor(
                            e3[:, chlo:chhi, :], e2[:, chlo:chhi, 0:25],
                            e2[:, chlo:chhi, 25:50], Alu.add)
                        nc.vector.tensor_reduce(
                            out_sb[:, ocol + chlo:ocol + chhi],
                            e3[:, chlo:chhi, :], Axis.X, Alu.add)
                    nc.vector.tensor_tensor(
                        r48[:], esb[:, 4000:4048], esb[:, 4048:4096],
                        Alu.add)
                    nc.vector.tensor_tensor(
                        r24[:], r48[:, 0:24], r48[:, 24:48], Alu.add)
                    nc.vector.tensor_reduce(
                        out_sb[:, ocol + NFULL:ocol + NCHUNK], r24[:],
                        Axis.X, Alu.add)
                if blk == NBLK - 2:  # hide most of the output writeback
                    nc.sync.dma_start(out.ap()[:, 0:(NBLK - 1) * NCHUNK],
                                      out_sb[:, 0:(NBLK - 1) * NCHUNK])

            nc.sync.dma_start(out.ap()[:, (NBLK - 1) * NCHUNK:],
                              out_sb[:, (NBLK - 1) * NCHUNK:])

    nc.compile()
    return nc


_NC_CACHE = None


def _get_nc():
    global _NC_CACHE
    if _NC_CACHE is None:
        _NC_CACHE = _build_kernel()
    return _NC_CACHE


# ---------------------------------------------------------------------------
# host orchestration: plan, gather, normalize, means, pos -> tiny device feeds
# ---------------------------------------------------------------------------
def _prep_inputs(input, input_logits, input_seg):
    x = np.asarray(input)
    plan = _plan(input_logits, input_seg)
    assert len(plan["g_anchor"]) == NA and len(plan["b_anchor"]) == NA
    assert plan["n_bg"] == NA

    x2d = x.reshape(C, HW)  # contiguous view, no copy

    pg_a = plan["g_anchor"][:, 1] * W + plan["g_anchor"][:, 2]
    pb_a = plan["b_anchor"][:, 1] * W + plan["b_anchor"][:, 2]
    pg_c = plan["g_core"][:, 1] * W + plan["g_core"][:, 2]
    pb_c = plan["b_core"][:, 1] * W + plan["b_core"][:, 2]
    ngc, nbc = len(pg_c), len(pb_c)

    # one gather for everything we need from x: [256, 2*NA + ngc + nbc]
    cols = np.concatenate([pg_a, pb_a, pg_c, pb_c])
    g = x2d[:, cols]
    nrm = np.sqrt(np.einsum("cp,cp->p", g, g, dtype=np.float32))
    gn = g / np.maximum(nrm, _EPS_NORM)[None, :]

    anc = gn[:, :2 * NA]                       # [C, 8192] normalized anchors
    mg = gn[:, 2 * NA:2 * NA + ngc].mean(axis=1)
    mb = gn[:, 2 * NA + ngc:].mean(axis=1)
    mgh = mg / max(np.sqrt(mg @ mg), 1e-8)
    mbh = mb / max(np.sqrt(mb @ mb), 1e-8)

    pos_g = anc[:, :NA].T @ mgh                # [NA]
    pos_b = anc[:, NA:].T @ mbh
    epos_all = np.exp(np.concatenate([pos_g, pos_b]) * (-1.0 / TAU)) \
        .astype(np.float32)

    anc_bf = (anc * SCALE).astype(F8_NP)
    ball_np = np.empty((2, 2, 128, NA), F8_NP)
    for h in range(2):
        for cls in range(2):
            ball_np[h, cls] = anc_bf[h * 128:(h + 1) * 128,
                                     cls * NA:(cls + 1) * NA]

    in_maps = []
    for k in range(N_CORES):
        amy_np = np.empty((2, 128, 2 * ASL), F8_NP)
        for h in range(2):
            for cls in range(2):
                amy_np[h, :, cls * ASL:(cls + 1) * ASL] = \
                    ball_np[h, cls][:, k * ASL:(k + 1) * ASL]
        in_maps.append({"amy": amy_np, "ball": ball_np})
    return in_maps, epos_all


def kernel(input, input_logits, input_seg):
    nc = _get_nc()
    in_maps, epos_all = _prep_inputs(input, input_logits, input_seg)
    res = run_bass_kernel_spmd(nc, in_maps, list(range(N_CORES)))
    tot = 0.0
    for k in range(N_CORES):
        r = res.results[k]["out"].astype(np.float32) \
            .reshape(128, NBLK, NCHUNK)
        for cls in range(2):
            for ib in range(NIB):
                lo = cls * NA + k * ASL + ib * 128
                sprime = r[:, cls * NIB + ib, :] * epos_all[lo:lo + 128, None]
                tot += np.log1p(sprime, dtype=np.float64).sum()
    loss = tot / (NCHUNK * NA)
    return np.float32(loss)
